# revision 11
# baseline (speedup 1.0000x reference)
"""Single-head causal attention (prefill) on 8 Trainium2 NeuronCores.

Problem: x[4,4096,2048], Wq/Wk/Wv[2048,128] -> out[4,4096,128]
  Q=xWq K=xWk V=xWv; out = softmax(mask(QK^T/sqrt(128))) V

Sharding: data-parallel over batch (4) x flash-style 2-way KEY split.
Core c handles batch c//2; half h=c%2 OWNS the 256-wide key blocks with
(key//256)%2 == h (2048 keys).  Each core:
  - projects K,V only for its owned keys (the big dedup win vs. splitting
    queries: no duplicated K/V projection across the pair),
  - projects Q for all 4096 queries,
  - computes partial attention numerator num[g] = sum_k e(k,q) V[k] and
    partial denominator den[g] = sum_k e(k,q) over its OWNED keys only,
    for every 512-query group g (causal: group g sees 2(g+1) owned
    128-key chunks; the last 2 are the in-group diagonal, masked on-device
    via threshold data: e = (iota_q >= thr_k) * exp(score)).
Host combines: out = (num_even + num_odd) / (den_even + den_odd).

All matmul operands are bf16 (fp32 PSUM accumulate): same PE streaming
rate as fp32r but half the DMA traffic, half the DVE element cost, and
fast-weight-load halves LDWEIGHTS so it hides behind the matmuls.

On-device per core (all N=512-col matmuls unless noted):
  proj KV: 4 tiles x 16 e-chunks x {K,V}      = 128 MM
  proj Q:  8 tiles x 16 e-chunks              = 128 MM
  scores:  sum_g 2(g+1) chunk MMs             =  72 MM
  attnV:   same                               =  72 MM
  den:     ones[128,1]^T @ (4-chunk e sums)   =  20 MM (M=1)
  V transposes (PE, 128x128 bf16)             =  16
"""

import numpy as np

B, T, E, D = 4, 4096, 2048, 128
NCORES = 8
G = 512                     # query-group width (PSUM bank = 512 fp32)
NG = T // G                 # 8 query groups
EC = E // 128               # 16 e-chunks
OK = T // 2                 # 2048 owned keys per core
HB = 256                    # ownership half-block width
SCALE = 1.0 / float(np.sqrt(D))

_CACHE = {}


def _emit(nc, tc, ctx, aps):
    import concourse.bass as bass  # noqa: F401
    from concourse import mybir

    f32 = mybir.dt.float32
    bf16 = mybir.dt.bfloat16
    xq, xkv, wq, wk, wv, thr, ident, num, den = (
        aps["xq"], aps["xkv"], aps["wq"], aps["wk"], aps["wv"], aps["thr"],
        aps["ident"], aps["num"], aps["den"],
    )

    # ---- pools ----
    wpool = ctx.enter_context(tc.tile_pool(name="w", bufs=1))
    cpool = ctx.enter_context(tc.tile_pool(name="const", bufs=1))
    xpool = ctx.enter_context(tc.tile_pool(name="xt", bufs=20))
    kpool = ctx.enter_context(tc.tile_pool(name="kt", bufs=1))
    vtpool = ctx.enter_context(tc.tile_pool(name="vt", bufs=2))
    qtpool = ctx.enter_context(tc.tile_pool(name="qt", bufs=3))
    epool = ctx.enter_context(tc.tile_pool(name="e", bufs=10))
    espool = ctx.enter_context(tc.tile_pool(name="es", bufs=4))
    opool_sb = ctx.enter_context(tc.tile_pool(name="osb", bufs=2))
    dpool_sb = ctx.enter_context(tc.tile_pool(name="dsb", bufs=1))

    ppool = ctx.enter_context(tc.tile_pool(name="pp", bufs=2, space="PSUM"))
    spool = ctx.enter_context(tc.tile_pool(name="sp", bufs=3, space="PSUM"))
    apool = ctx.enter_context(tc.tile_pool(name="av", bufs=2, space="PSUM"))
    denp = ctx.enter_context(tc.tile_pool(name="den", bufs=1, space="PSUM"))

    # ---- constants (gpsimd DMA queue; x streams on sync, outputs on scalar)
    w_sb = {}
    for name, ap in (("wq", wq), ("wk", wk), ("wv", wv)):
        w_sb[name] = wpool.tile([128, EC, 128], bf16, tag=name, name=name)
    for r in range(0, EC, 4):
        for name, ap in (("wk", wk), ("wv", wv), ("wq", wq)):
            nc.gpsimd.dma_start(
                out=w_sb[name][:, r:r + 4, :], in_=ap[:, r:r + 4, :])
    identity = cpool.tile([128, 128], bf16, tag="ident", name="ident")
    nc.gpsimd.dma_start(out=identity[:, :], in_=ident[:, :])
    thr_sb = cpool.tile([128, 2], f32, tag="thr", name="thr")
    nc.gpsimd.dma_start(out=thr_sb[:, :], in_=thr[:, :])
    ones = cpool.tile([128, 1], bf16, tag="ones", name="ones")
    nc.vector.memset(ones[:, :], 1.0)
    iota = cpool.tile([128, G], f32, tag="iota", name="iota")
    nc.gpsimd.iota(
        iota[:, :], pattern=[[1, G]], base=0, channel_multiplier=0,
        allow_small_or_imprecise_dtypes=True,
    )
    # Warm the PE HAM clock gate while the first x/W DMAs stream.
    wsc = cpool.tile([128, G], bf16, tag="wsc", name="wsc")
    nc.vector.memset(wsc[:, :], 1.0)
    wps = spool.tile([128, G], f32, tag="sp", name="wps")
    for _ in range(12):
        nc.tensor.matmul(
            wps[:, :], wsc[:, 0:128], wsc[:, :], start=True, stop=True)

    KT = kpool.tile([128, OK], bf16, tag="KT", name="KT")
    VN = kpool.tile([128, OK], bf16, tag="VN", name="VN")
    QT = [None] * NG

    xq_r = xq.rearrange("(c p) t -> p c t", p=128)
    xkv_r = xkv.rearrange("(c p) t -> p c t", p=128)

    def proj_kv_units(j):
        """K,V projection of owned-key tile j, as 4 yieldable units."""
        pk = ppool.tile([128, G], f32, tag="pp", name="pk")
        pv = ppool.tile([128, G], f32, tag="pp", name="pv")
        for q4 in range(4):
            xt = xpool.tile([128, 4, G], bf16, tag="xt", name="xkv")
            nc.sync.dma_start(
                out=xt[:, :, :],
                in_=xkv_r[:, q4 * 4:(q4 + 1) * 4, j * G:(j + 1) * G],
            )
            for i in range(4):
                jj = q4 * 4 + i
                rhs = xt[:, i, :]
                st, sp = jj == 0, jj == EC - 1
                nc.tensor.matmul(
                    pk[:, :], w_sb["wk"][:, jj, :], rhs, start=st, stop=sp)
                nc.tensor.matmul(
                    pv[:, :], w_sb["wv"][:, jj, :], rhs, start=st, stop=sp)
            if q4 < 3:
                yield
        nc.scalar.copy(out=KT[:, j * G:(j + 1) * G], in_=pk[:, :])
        vt = vtpool.tile([128, G], bf16, tag="vt", name="vt")
        nc.scalar.copy(out=vt[:, :], in_=pv[:, :])
        for c in range(4):
            pt = spool.tile([128, 128], bf16, tag="sp", name="tp")
            nc.tensor.transpose(
                pt[:, :], vt[:, c * 128:(c + 1) * 128], identity[:, :])
            nc.vector.tensor_copy(
                out=VN[:, (j * 4 + c) * 128:(j * 4 + c + 1) * 128],
                in_=pt[:, :])

    def proj_kv(j):
        for _ in proj_kv_units(j):
            pass

    def proj_q(g):
        """Q projection of query group g (all queries)."""
        pq = ppool.tile([128, G], f32, tag="pp", name="pq")
        for q4 in range(4):
            xt = xpool.tile([128, 4, G], bf16, tag="xt", name="xq")
            nc.sync.dma_start(
                out=xt[:, :, :],
                in_=xq_r[:, q4 * 4:(q4 + 1) * 4, g * G:(g + 1) * G],
            )
            for i in range(4):
                jj = q4 * 4 + i
                st, sp = jj == 0, jj == EC - 1
                nc.tensor.matmul(
                    pq[:, :], w_sb["wq"][:, jj, :], xt[:, i, :],
                    start=st, stop=sp)
        qt = qtpool.tile([128, G], bf16, tag="qt", name="qt")
        nc.scalar.copy(out=qt[:, :], in_=pq[:, :])
        QT[g] = qt

    den_sb = dpool_sb.tile([1, NG * G], f32, tag="den", name="den_sb")

    LAG = 2   # attnV trails scores by LAG chunks to hide the exp latency

    class Attn:
        """Partial attention of query group g over the owned-key prefix,
        software-pipelined: attnV/den consume chunk c-LAG while scores/exp
        of chunk c stream, so the PE never head-of-line blocks on ACT."""

        def __init__(self, g):
            self.g = g
            self.nch = 2 * (g + 1)
            self.po = apool.tile([128, G], f32, tag="av", name="po")
            self.e = [None] * self.nch
            self.e_acc = None
            self.next_av = 0

        def score(self, c):
            ps = spool.tile([128, G], f32, tag="sp", name="ps")
            nc.tensor.matmul(
                ps[:, :], KT[:, c * 128:(c + 1) * 128], QT[self.g][:, :],
                start=True, stop=True)
            e = epool.tile([128, G], bf16, tag="e", name="e")
            nc.scalar.activation(
                out=e[:, :], in_=ps[:, :],
                func=mybir.ActivationFunctionType.Exp, scale=SCALE,
            )
            mi = c - (self.nch - 2)
            if mi >= 0:
                nc.vector.scalar_tensor_tensor(
                    out=e[:, :],
                    in0=iota[:, :],
                    scalar=thr_sb[:, mi:mi + 1],
                    in1=e[:, :],
                    op0=mybir.AluOpType.is_ge,
                    op1=mybir.AluOpType.mult,
                )
            self.e[c] = e

        def av(self, c):
            e = self.e[c]
            nc.tensor.matmul(
                self.po[:, :], VN[:, c * 128:(c + 1) * 128], e[:, :],
                start=(c == 0), stop=(c == self.nch - 1))
            if c == 0:
                self.e_acc = e
            else:
                es = espool.tile([128, G], bf16, tag="es", name="es")
                nc.vector.tensor_add(es[:, :], self.e_acc[:, :], e[:, :])
                self.e_acc = es
            self.e[c] = None

        def chunk(self, c):
            self.score(c)
            if c >= LAG:
                self.av(c - LAG)
                self.next_av = c - LAG + 1

        def flush(self):
            g, nch = self.g, self.nch
            for c in range(self.next_av, nch):
                self.av(c)
            pd = denp.tile([1, G], f32, tag="den", name="pd")
            nc.tensor.matmul(
                pd[:, :], ones[:, :], self.e_acc[:, :], start=True, stop=True)
            osb = opool_sb.tile([128, G], f32, tag="osb", name="osb")
            nc.vector.tensor_copy(out=osb[:, :], in_=self.po[:, :])
            nc.scalar.dma_start(out=num[g, :, :], in_=osb[:, :])
            nc.vector.tensor_copy(
                out=den_sb[:, g * G:(g + 1) * G], in_=pd[:, :])

        def run(self, c0, c1):
            for c in range(c0, c1):
                self.chunk(c)
            if c1 == self.nch:
                self.flush()

    def attn(g):
        a = Attn(g)
        a.run(0, a.nch)

    # Schedule: stream projections (DMA-hungry) early; attention (DMA-free)
    # as soon as its KV prefix + Q group are resident.  KV tile 3 is
    # co-emitted with attn(6)/attn(7) chunks 0-11 (which only need KV
    # tiles 0-2) so the final stretch isn't a bare scores->exp->attnV
    # dependency chain; the two diagonal tails interleave for the same
    # reason.
    proj_kv(0)
    proj_q(0)
    attn(0)
    proj_q(1)
    attn(1)
    proj_kv(1)
    proj_q(2)
    attn(2)
    proj_q(3)
    attn(3)
    proj_kv(2)
    proj_q(4)
    attn(4)
    proj_q(5)
    attn(5)
    proj_q(6)
    proj_q(7)
    a6, a7 = Attn(6), Attn(7)
    kv3 = proj_kv_units(3)
    for q4 in range(4):
        next(kv3, None)
        a6.run(q4 * 3, q4 * 3 + 3)
        a7.run(q4 * 3, q4 * 3 + 3)
    nc.scalar.dma_start(out=den[:, 0:6 * G], in_=den_sb[:, 0:6 * G])
    for c in (12, 13):
        a6.chunk(c)
        a7.chunk(c)
    a7.chunk(14)
    a7.chunk(15)
    a6.flush()
    a7.flush()
    nc.scalar.dma_start(out=den[:, 6 * G:], in_=den_sb[:, 6 * G:])


def _build():
    if "nc" in _CACHE:
        return _CACHE["nc"]
    from contextlib import ExitStack

    import concourse.bacc as bacc
    import concourse.tile as tile
    from concourse import mybir

    f32 = mybir.dt.float32
    bf16 = mybir.dt.bfloat16
    nc = bacc.Bacc(
        "TRN2", target_bir_lowering=False, debug=False, enable_asserts=False,
        num_devices=NCORES,
    )
    aps = {
        "xq": nc.dram_tensor("xq", [E, T], bf16, kind="ExternalInput").ap(),
        "xkv": nc.dram_tensor("xkv", [E, OK], bf16, kind="ExternalInput").ap(),
        "wq": nc.dram_tensor("wq", [128, EC, D], bf16, kind="ExternalInput").ap(),
        "wk": nc.dram_tensor("wk", [128, EC, D], bf16, kind="ExternalInput").ap(),
        "wv": nc.dram_tensor("wv", [128, EC, D], bf16, kind="ExternalInput").ap(),
        "thr": nc.dram_tensor("thr", [128, 2], f32, kind="ExternalInput").ap(),
        "ident": nc.dram_tensor(
            "ident", [128, 128], bf16, kind="ExternalInput").ap(),
        "num": nc.dram_tensor(
            "num", [NG, 128, G], f32, kind="ExternalOutput").ap(),
        "den": nc.dram_tensor(
            "den", [1, NG * G], f32, kind="ExternalOutput").ap(),
    }
    with tile.TileContext(nc) as tc, ExitStack() as ctx:
        _emit(nc, tc, ctx, aps)
    nc.compile()
    _CACHE["nc"] = nc
    return nc


def make_in_maps(x, Wq, Wk, Wv):
    import ml_dtypes

    bf = ml_dtypes.bfloat16
    x = np.asarray(x, dtype=np.float32)

    def wshape(W):
        # [E, D] -> [128, EC, D]: chunk c rows c*128..c*128+127 at [:, c, :]
        return np.ascontiguousarray(
            np.asarray(W, dtype=np.float32).reshape(EC, 128, D)
            .transpose(1, 0, 2).astype(bf))

    common = {
        "wq": wshape(Wq), "wk": wshape(Wk), "wv": wshape(Wv),
        "ident": np.eye(128, dtype=np.float32).astype(bf),
    }
    row = np.arange(128, dtype=np.float32)
    thrs = [
        np.stack([256.0 * h + row, 256.0 * h + 128.0 + row], axis=1)
        .astype(np.float32)
        for h in range(2)
    ]
    idx = np.arange(T)
    sel = [idx[(idx // HB) % 2 == h] for h in range(2)]
    in_maps = []
    xq_b = {}
    for c in range(NCORES):
        b, h = c // 2, c % 2
        if b not in xq_b:
            xq_b[b] = np.ascontiguousarray(x[b].T.astype(bf))     # [E, T]
        xkv = np.ascontiguousarray(x[b][sel[h]].T.astype(bf))     # [E, OK]
        in_maps.append(
            {**common, "xq": xq_b[b], "xkv": xkv, "thr": thrs[h]})
    return in_maps


def gather(results):
    out = np.empty((B, T, D), dtype=np.float32)
    for b in range(B):
        rE, rO = results[2 * b], results[2 * b + 1]
        nsum = rE["num"] + rO["num"]                  # [NG, 128, G]
        dsum = (rE["den"] + rO["den"]).reshape(NG, 1, G)
        out[b] = (nsum / dsum).transpose(0, 2, 1).reshape(T, D)
    return out


def run(x, Wq, Wk, Wv, trace=False, **trace_kwargs):
    from concourse.bass_utils import run_bass_kernel_spmd

    nc = _build()
    in_maps = make_in_maps(x, Wq, Wk, Wv)
    res = run_bass_kernel_spmd(
        nc, in_maps, core_ids=list(range(NCORES)), trace=trace, **trace_kwargs)
    return gather(res.results), res


def kernel(x, Wq, Wk, Wv):
    out, _ = run(np.asarray(x), np.asarray(Wq), np.asarray(Wk), np.asarray(Wv))
    return out


# revision 15
# speedup vs baseline: 1.1130x; 1.1130x over previous
"""Single-head causal attention (prefill) on 8 Trainium2 NeuronCores.

Problem: x[4,4096,2048], Wq/Wk/Wv[2048,128] -> out[4,4096,128]
  Q=xWq K=xWk V=xWv; out = softmax(mask(QK^T/sqrt(128))) V

Sharding: data-parallel over batch (4) x flash-style 2-way KEY split.
Core c handles batch c//2; half h=c%2 OWNS the 256-wide key blocks with
(key//256)%2 == h (2048 keys).  Each core:
  - projects K,V only for its owned keys (the big dedup win vs. splitting
    queries: no duplicated K/V projection across the pair),
  - projects Q for all 4096 queries,
  - computes partial attention numerator num[g] = sum_k e(k,q) V[k] and
    partial denominator den[g] = sum_k e(k,q) over its OWNED keys only,
    for every 512-query group g (causal: group g sees 2(g+1) owned
    128-key chunks; the last 2 are the in-group diagonal, masked on-device
    via threshold data: e = (iota_q >= thr_k) * exp(score)).
Host combines: out = (num_even + num_odd) / (den_even + den_odd).

All matmul operands are bf16 (fp32 PSUM accumulate): same PE streaming
rate as fp32r but half the DMA traffic, half the DVE element cost, and
fast-weight-load halves LDWEIGHTS so it hides behind the matmuls.

On-device per core (all N=512-col matmuls unless noted):
  proj KV: 4 tiles x 16 e-chunks x {K,V}      = 128 MM
  proj Q:  8 tiles x 16 e-chunks              = 128 MM
  scores:  sum_g 2(g+1) chunk MMs             =  72 MM
  attnV:   same                               =  72 MM
  den:     ones[128,1]^T @ (4-chunk e sums)   =  20 MM (M=1)
  V transposes (PE, 128x128 bf16)             =  16
"""

import numpy as np

B, T, E, D = 4, 4096, 2048, 128
NCORES = 8
G = 512                     # query-group width (PSUM bank = 512 fp32)
NG = T // G                 # 8 query groups
EC = E // 128               # 16 e-chunks
OK = T // 2                 # 2048 owned keys per core
HB = 256                    # ownership half-block width
SCALE = 1.0 / float(np.sqrt(D))

_CACHE = {}


def _emit(nc, tc, ctx, aps):
    import concourse.bass as bass  # noqa: F401
    from concourse import mybir

    f32 = mybir.dt.float32
    bf16 = mybir.dt.bfloat16
    xq, xkv, wq, wk, wv, thr, ident, num, den = (
        aps["xq"], aps["xkv"], aps["wq"], aps["wk"], aps["wv"], aps["thr"],
        aps["ident"], aps["num"], aps["den"],
    )

    # ---- pools ----
    wpool = ctx.enter_context(tc.tile_pool(name="w", bufs=1))
    cpool = ctx.enter_context(tc.tile_pool(name="const", bufs=1))
    xpool = ctx.enter_context(tc.tile_pool(name="xt", bufs=20))
    kpool = ctx.enter_context(tc.tile_pool(name="kt", bufs=1))
    vtpool = ctx.enter_context(tc.tile_pool(name="vt", bufs=2))
    qtpool = ctx.enter_context(tc.tile_pool(name="qt", bufs=3))
    epool = ctx.enter_context(tc.tile_pool(name="e", bufs=10))
    espool = ctx.enter_context(tc.tile_pool(name="es", bufs=4))
    opool_sb = ctx.enter_context(tc.tile_pool(name="osb", bufs=2))
    dpool_sb = ctx.enter_context(tc.tile_pool(name="dsb", bufs=1))

    ppool = ctx.enter_context(tc.tile_pool(name="pp", bufs=2, space="PSUM"))
    spool = ctx.enter_context(tc.tile_pool(name="sp", bufs=3, space="PSUM"))
    apool = ctx.enter_context(tc.tile_pool(name="av", bufs=2, space="PSUM"))
    denp = ctx.enter_context(tc.tile_pool(name="den", bufs=1, space="PSUM"))

    # ---- constants (gpsimd DMA queue; x streams on sync, outputs on scalar)
    w_sb = {}
    for name, ap in (("wq", wq), ("wk", wk), ("wv", wv)):
        w_sb[name] = wpool.tile([128, EC, 128], bf16, tag=name, name=name)
    for r in range(0, EC, 4):
        for name, ap in (("wk", wk), ("wv", wv), ("wq", wq)):
            nc.gpsimd.dma_start(
                out=w_sb[name][:, r:r + 4, :], in_=ap[:, r:r + 4, :])
    identity = cpool.tile([128, 128], bf16, tag="ident", name="ident")
    nc.gpsimd.dma_start(out=identity[:, :], in_=ident[:, :])
    thr_sb = cpool.tile([128, 2], f32, tag="thr", name="thr")
    nc.gpsimd.dma_start(out=thr_sb[:, :], in_=thr[:, :])
    ones = cpool.tile([128, 1], bf16, tag="ones", name="ones")
    nc.vector.memset(ones[:, :], 1.0)
    iota = cpool.tile([128, G], f32, tag="iota", name="iota")
    nc.gpsimd.iota(
        iota[:, :], pattern=[[1, G]], base=0, channel_multiplier=0,
        allow_small_or_imprecise_dtypes=True,
    )
    # Warm the PE HAM clock gate while the first x/W DMAs stream.
    wsc = cpool.tile([128, G], bf16, tag="wsc", name="wsc")
    nc.gpsimd.memset(wsc[:, :], 1.0)
    wps = spool.tile([128, G], f32, tag="sp", name="wps")
    for _ in range(12):
        nc.tensor.matmul(
            wps[:, :], wsc[:, 0:128], wsc[:, :], start=True, stop=True)

    KT = kpool.tile([128, OK], bf16, tag="KT", name="KT")
    VN = kpool.tile([128, OK], bf16, tag="VN", name="VN")
    QT = [None] * NG

    xq_r = xq.rearrange("(c p) t -> p c t", p=128)
    xkv_r = xkv.rearrange("(c p) t -> p c t", p=128)

    def proj_kv_units(j):
        """K,V projection of owned-key tile j, as 4 yieldable units."""
        pk = ppool.tile([128, G], f32, tag="pp", name="pk")
        pv = ppool.tile([128, G], f32, tag="pp", name="pv")
        for q4 in range(4):
            xt = xpool.tile([128, 4, G], bf16, tag="xt", name="xkv")
            nc.sync.dma_start(
                out=xt[:, :, :],
                in_=xkv_r[:, q4 * 4:(q4 + 1) * 4, j * G:(j + 1) * G],
            )
            for i in range(4):
                jj = q4 * 4 + i
                rhs = xt[:, i, :]
                st, sp = jj == 0, jj == EC - 1
                nc.tensor.matmul(
                    pk[:, :], w_sb["wk"][:, jj, :], rhs, start=st, stop=sp)
                nc.tensor.matmul(
                    pv[:, :], w_sb["wv"][:, jj, :], rhs, start=st, stop=sp)
            if q4 < 3:
                yield
        nc.scalar.copy(out=KT[:, j * G:(j + 1) * G], in_=pk[:, :])
        vt = vtpool.tile([128, G], bf16, tag="vt", name="vt")
        nc.scalar.copy(out=vt[:, :], in_=pv[:, :])
        for c in range(4):
            pt = spool.tile([128, 128], bf16, tag="sp", name="tp")
            nc.tensor.transpose(
                pt[:, :], vt[:, c * 128:(c + 1) * 128], identity[:, :])
            nc.vector.tensor_copy(
                out=VN[:, (j * 4 + c) * 128:(j * 4 + c + 1) * 128],
                in_=pt[:, :])

    def proj_kv(j):
        for _ in proj_kv_units(j):
            pass

    def proj_q(g):
        """Q projection of query group g (all queries)."""
        pq = ppool.tile([128, G], f32, tag="pp", name="pq")
        for q4 in range(4):
            xt = xpool.tile([128, 4, G], bf16, tag="xt", name="xq")
            nc.sync.dma_start(
                out=xt[:, :, :],
                in_=xq_r[:, q4 * 4:(q4 + 1) * 4, g * G:(g + 1) * G],
            )
            for i in range(4):
                jj = q4 * 4 + i
                st, sp = jj == 0, jj == EC - 1
                nc.tensor.matmul(
                    pq[:, :], w_sb["wq"][:, jj, :], xt[:, i, :],
                    start=st, stop=sp)
        qt = qtpool.tile([128, G], bf16, tag="qt", name="qt")
        nc.scalar.copy(out=qt[:, :], in_=pq[:, :])
        QT[g] = qt

    den_sb = dpool_sb.tile([1, NG * G], f32, tag="den", name="den_sb")

    LAG = 3   # attnV trails scores by LAG chunks to hide the exp latency

    class Attn:
        """Partial attention of query group g over the owned-key prefix,
        software-pipelined: attnV/den consume chunk c-LAG while scores/exp
        of chunk c stream, so the PE never head-of-line blocks on ACT."""

        def __init__(self, g):
            self.g = g
            self.nch = 2 * (g + 1)
            self.po = apool.tile([128, G], f32, tag="av", name="po")
            self.e = [None] * self.nch
            self.e_acc = None
            self.next_av = 0

        def score(self, c):
            ps = spool.tile([128, G], f32, tag="sp", name="ps")
            nc.tensor.matmul(
                ps[:, :], KT[:, c * 128:(c + 1) * 128], QT[self.g][:, :],
                start=True, stop=True)
            e = epool.tile([128, G], bf16, tag="e", name="e")
            nc.scalar.activation(
                out=e[:, :], in_=ps[:, :],
                func=mybir.ActivationFunctionType.Exp, scale=SCALE,
            )
            mi = c - (self.nch - 2)
            if mi >= 0:
                nc.vector.scalar_tensor_tensor(
                    out=e[:, :],
                    in0=iota[:, :],
                    scalar=thr_sb[:, mi:mi + 1],
                    in1=e[:, :],
                    op0=mybir.AluOpType.is_ge,
                    op1=mybir.AluOpType.mult,
                )
            self.e[c] = e

        def av(self, c):
            e = self.e[c]
            nc.tensor.matmul(
                self.po[:, :], VN[:, c * 128:(c + 1) * 128], e[:, :],
                start=(c == 0), stop=(c == self.nch - 1))
            if c == 0:
                self.e_acc = e
            else:
                es = espool.tile([128, G], bf16, tag="es", name="es")
                nc.vector.tensor_add(es[:, :], self.e_acc[:, :], e[:, :])
                self.e_acc = es
            self.e[c] = None

        def chunk(self, c):
            self.score(c)
            if c >= LAG:
                self.av(c - LAG)
                self.next_av = c - LAG + 1

        def finish(self):
            g = self.g
            pd = denp.tile([1, G], f32, tag="den", name="pd")
            nc.tensor.matmul(
                pd[:, :], ones[:, :], self.e_acc[:, :], start=True, stop=True)
            osb = opool_sb.tile([128, G], f32, tag="osb", name="osb")
            nc.vector.tensor_copy(out=osb[:, :], in_=self.po[:, :])
            nc.scalar.dma_start(out=num[g, :, :], in_=osb[:, :])
            nc.vector.tensor_copy(
                out=den_sb[:, g * G:(g + 1) * G], in_=pd[:, :])

        def flush(self):
            for c in range(self.next_av, self.nch):
                self.av(c)
            self.finish()

        def run(self, c0, c1):
            for c in range(c0, c1):
                self.chunk(c)
            if c1 == self.nch:
                self.flush()

    def attn(g):
        a = Attn(g)
        a.run(0, a.nch)

    # Schedule: stream projections (DMA-hungry) early; attention (DMA-free)
    # as soon as its KV prefix + Q group are resident.  KV tile 3 is
    # co-emitted with attn(6)/attn(7) chunks 0-11 (which only need KV
    # tiles 0-2) so the final stretch isn't a bare scores->exp->attnV
    # dependency chain; the two diagonal tails interleave for the same
    # reason.
    import itertools

    proj_kv(0)
    proj_q(0)
    proj_q(1)
    attn(0)
    proj_q(2)
    attn(1)
    proj_kv(1)
    proj_q(3)
    attn(2)
    proj_q(4)
    attn(3)
    proj_kv(2)
    proj_q(5)
    attn(4)
    proj_q(6)
    attn(5)
    proj_q(7)
    a6, a7 = Attn(6), Attn(7)
    kv3 = proj_kv_units(3)
    for q4 in range(4):
        next(kv3, None)
        a6.run(q4 * 3, q4 * 3 + 3)
        a7.run(q4 * 3, q4 * 3 + 3)
    nc.scalar.dma_start(out=den[:, 0:6 * G], in_=den_sb[:, 0:6 * G])
    for c in (12, 13):
        a6.chunk(c)
        a7.chunk(c)
    a7.chunk(14)
    a7.chunk(15)
    for c6, c7 in itertools.zip_longest(
            range(a6.next_av, a6.nch), range(a7.next_av, a7.nch)):
        if c6 is not None:
            a6.av(c6)
        if c7 is not None:
            a7.av(c7)
    a6.finish()
    a7.finish()
    nc.scalar.dma_start(out=den[:, 6 * G:], in_=den_sb[:, 6 * G:])


def _build():
    if "nc" in _CACHE:
        return _CACHE["nc"]
    from contextlib import ExitStack

    import concourse.bacc as bacc
    import concourse.tile as tile
    from concourse import mybir

    f32 = mybir.dt.float32
    bf16 = mybir.dt.bfloat16
    nc = bacc.Bacc(
        "TRN2", target_bir_lowering=False, debug=False, enable_asserts=False,
        num_devices=NCORES,
    )
    aps = {
        "xq": nc.dram_tensor("xq", [E, T], bf16, kind="ExternalInput").ap(),
        "xkv": nc.dram_tensor("xkv", [E, OK], bf16, kind="ExternalInput").ap(),
        "wq": nc.dram_tensor("wq", [128, EC, D], bf16, kind="ExternalInput").ap(),
        "wk": nc.dram_tensor("wk", [128, EC, D], bf16, kind="ExternalInput").ap(),
        "wv": nc.dram_tensor("wv", [128, EC, D], bf16, kind="ExternalInput").ap(),
        "thr": nc.dram_tensor("thr", [128, 2], f32, kind="ExternalInput").ap(),
        "ident": nc.dram_tensor(
            "ident", [128, 128], bf16, kind="ExternalInput").ap(),
        "num": nc.dram_tensor(
            "num", [NG, 128, G], f32, kind="ExternalOutput").ap(),
        "den": nc.dram_tensor(
            "den", [1, NG * G], f32, kind="ExternalOutput").ap(),
    }
    with tile.TileContext(nc) as tc, ExitStack() as ctx:
        _emit(nc, tc, ctx, aps)
    nc.compile()
    _CACHE["nc"] = nc
    return nc


def make_in_maps(x, Wq, Wk, Wv):
    import ml_dtypes

    bf = ml_dtypes.bfloat16
    x = np.asarray(x, dtype=np.float32)

    def wshape(W):
        # [E, D] -> [128, EC, D]: chunk c rows c*128..c*128+127 at [:, c, :]
        return np.ascontiguousarray(
            np.asarray(W, dtype=np.float32).reshape(EC, 128, D)
            .transpose(1, 0, 2).astype(bf))

    common = {
        "wq": wshape(Wq), "wk": wshape(Wk), "wv": wshape(Wv),
        "ident": np.eye(128, dtype=np.float32).astype(bf),
    }
    row = np.arange(128, dtype=np.float32)
    thrs = [
        np.stack([256.0 * h + row, 256.0 * h + 128.0 + row], axis=1)
        .astype(np.float32)
        for h in range(2)
    ]
    idx = np.arange(T)
    sel = [idx[(idx // HB) % 2 == h] for h in range(2)]
    in_maps = []
    xq_b = {}
    for c in range(NCORES):
        b, h = c // 2, c % 2
        if b not in xq_b:
            xq_b[b] = np.ascontiguousarray(x[b].T.astype(bf))     # [E, T]
        xkv = np.ascontiguousarray(x[b][sel[h]].T.astype(bf))     # [E, OK]
        in_maps.append(
            {**common, "xq": xq_b[b], "xkv": xkv, "thr": thrs[h]})
    return in_maps


def gather(results):
    out = np.empty((B, T, D), dtype=np.float32)
    for b in range(B):
        rE, rO = results[2 * b], results[2 * b + 1]
        nsum = rE["num"] + rO["num"]                  # [NG, 128, G]
        dsum = (rE["den"] + rO["den"]).reshape(NG, 1, G)
        out[b] = (nsum / dsum).transpose(0, 2, 1).reshape(T, D)
    return out


def run(x, Wq, Wk, Wv, trace=False, **trace_kwargs):
    from concourse.bass_utils import run_bass_kernel_spmd

    nc = _build()
    in_maps = make_in_maps(x, Wq, Wk, Wv)
    res = run_bass_kernel_spmd(
        nc, in_maps, core_ids=list(range(NCORES)), trace=trace, **trace_kwargs)
    return gather(res.results), res


def kernel(x, Wq, Wk, Wv):
    out, _ = run(np.asarray(x), np.asarray(Wq), np.asarray(Wk), np.asarray(Wv))
    return out
